# revision 1
# baseline (speedup 1.0000x reference)
"""DifferentialAttention Trainium2 kernel, 8-core SPMD (tensor-parallel over heads).

Reference computation (B=2, T=2048, d_model=1024, H=16, D=64):
    qkv = x @ W_qkv                     -> q1,q2,k1,k2,v per head
    s_i = q_i k_i^T / sqrt(D) + causal_mask ; a_i = softmax(s_i)
    attn = a1 - clip(lam,0,1) * a2
    out  = (attn @ v) @ W_out

Sharding: 2 heads per core (16 heads / 8 cores). Each core computes the full
qkv projection for its heads' columns, the per-head attention, and a partial
out-projection (its 128 rows of W_out); host sums the 8 partial outputs.

Layout strategy (per core):
  - x is staged host-side transposed: xt[d_model, B*T], so the qkv projection
    runs as qkvT[c, i] = W_qkv_loc[:, c].T-contraction with xt as the moving
    operand, producing q/k in the transposed [head_dim, token] layout that the
    attention score matmuls consume directly (no on-device transposes of x/q/k).
  - Scores are computed transposed, sT[j, i] = (k_tile)^T-contract-q, so
    exp(sT) feeds the attn@v matmul as the moving operand with v (transposed
    on-device via the PE) as the stationary operand -- no p transposes.
  - Softmax normalizers Z[i] = sum_j exp(s[j,i]) are computed on the
    TensorEngine as ones-matrix matmuls with M=64, which also performs the
    partition broadcast: each score's Z bank holds 64 replicated rows per
    head, so a single DVE reciprocal yields the [128, i] scale tile.
  - Causality is exploited structurally: fully-masked j-tiles are skipped,
    diagonal 128-blocks get the (host-staged) triangular additive mask.
  - The qkv projection and score matmuls run in bf16 (f32 PSUM accumulation),
    attn@v and Z in bf16, the final out-projection in float32r (full-rate
    fp32); all verified against the f32 reference at ~4e-3 max relative error.
"""

import numpy as np

B, T, DM, H, D = 2, 2048, 1024, 16, 64
NCORES = 8
HL = H // NCORES          # heads per core = 2
NI = B * T                # 4096 token rows
IB = 512                  # i-block (query) width
JT = 128                  # j-tile (key) width
NIB = T // IB             # 4 i-blocks per batch
KO = DM // 128            # 8 contraction chunks

_cached = {}


def _modules():
    if "mods" in _cached:
        return _cached["mods"]
    import sys
    try:
        import concourse.bass  # noqa: F401
    except ImportError:
        sys.path.insert(0, "/opt/trn_rl_repo")
    import concourse.bass as bass
    import concourse.bacc as bacc
    import concourse.mybir as mybir
    import concourse.tile as tile
    from concourse.bass_utils import run_bass_kernel_spmd
    from concourse.masks import make_identity
    _cached["mods"] = (bacc, mybir, tile, run_bass_kernel_spmd, make_identity)
    return _cached["mods"]


def build_nc(reps=1):
    """Build the single-core Bass program (identical on all 8 cores).

    reps>1 repeats the whole computation back-to-back in one NEFF --
    used only for timing (per-rep delta cancels dispatch overhead)."""
    key = ("nc", reps)
    if key in _cached:
        return _cached[key]
    bacc, mybir, tile, _, make_identity = _modules()
    f32 = mybir.dt.float32
    f32r = mybir.dt.float32r
    bf16 = mybir.dt.bfloat16
    EXP = mybir.ActivationFunctionType.Exp
    MUL = mybir.AluOpType.mult
    SUB = mybir.AluOpType.subtract
    ADD = mybir.AluOpType.add

    nc = bacc.Bacc(None, target_bir_lowering=False, debug=False)

    xt = nc.dram_tensor("xt", [DM, NI], f32, kind="ExternalInput")
    wq = nc.dram_tensor("wq", [DM, 5 * 128], f32, kind="ExternalInput")
    wo = nc.dram_tensor("wo", [128, DM], f32, kind="ExternalInput")
    tri = nc.dram_tensor("tri", [128, 128], f32, kind="ExternalInput")
    lam128 = nc.dram_tensor("lam128", [128, 1], f32, kind="ExternalInput")
    y = nc.dram_tensor("y", [NI, DM], f32, kind="ExternalOutput")

    xt_r = xt.rearrange("(ko ki) n -> ki ko n", ki=128)
    wq_r = wq.rearrange("(ko ki) c -> ki ko c", ki=128)

    with tile.TileContext(nc) as tc:
        with (
            tc.tile_pool(name="const", bufs=1) as const,
            tc.tile_pool(name="xts", bufs=2) as xts_p,
            tc.tile_pool(name="vtmp", bufs=2) as vtmp_p,
            tc.tile_pool(name="pp", bufs=6) as pp_p,
            tc.tile_pool(name="rb", bufs=4) as rb_p,
            tc.tile_pool(name="of", bufs=2) as of_p,
            tc.tile_pool(name="tmp", bufs=2) as tmp_p,
            tc.tile_pool(name="ys", bufs=3) as ys_p,
            tc.tile_pool(name="psw", bufs=2, space="PSUM") as psw,
            tc.tile_pool(name="psa", bufs=3, space="PSUM") as psa,
            tc.tile_pool(name="psq", bufs=1, space="PSUM") as psq,
        ):
            # ---- static tiles
            WQ = const.tile([128, KO, 5 * 128], bf16)
            nc.gpsimd.dma_start(WQ[:], wq_r)
            WO = const.tile([128, DM], f32r)
            nc.sync.dma_start(WO[:], wo[:].bitcast(f32r))
            TRI = const.tile([128, 128], f32)
            nc.sync.dma_start(TRI[:], tri[:])
            LAMC = const.tile([128, 1], f32)
            nc.sync.dma_start(LAMC[:], lam128[:])
            nc.vector.tensor_scalar(
                LAMC[:], LAMC[:], 0.0, 1.0, mybir.AluOpType.max, mybir.AluOpType.min
            )
            IDN = const.tile([128, 128], f32)
            make_identity(nc, IDN[:])
            ONE = const.tile([128, 64], bf16)
            nc.vector.memset(ONE[:], 1.0)
            # persistent activations: q1/q2/k1/k2 chunks (c=0..3) and v-natural
            QKS = const.tile([128, 4, NI], bf16)      # [dim(A|B), chunk, token]
            VN = const.tile([128, B * 16, 128], bf16)  # [token%128, jtile, dim(A|B)]

            def make_qkv_emitters(b, ib):
                """Per-chunk emission closures for pair (b, ib), so the next
                i-block's projection interleaves into the current attention
                loop (keeps ACT fed while PE fills exp-wait gaps)."""
                i0 = b * T + ib * IB
                state = {}

                def dma():
                    xts = xts_p.tile([128, KO, IB], bf16)
                    nc.gpsimd.dma_start(xts[:], xt_r[:, :, i0:i0 + IB])
                    state["xts"] = xts

                ems = [dma]

                def chunk(c):
                    xts = state["xts"]
                    ps = psq.tile([128, IB], f32, tag="q")
                    pslice = ps[:, 0:IB]
                    for k in range(KO):
                        nc.tensor.matmul(
                            pslice,
                            WQ[:, k, c * 128:(c + 1) * 128],
                            xts[:, k, :],
                            start=(k == 0),
                            stop=(k == KO - 1),
                        )
                    if c < 4:
                        nc.vector.tensor_copy(QKS[:, c, i0:i0 + IB], pslice)
                    else:
                        vts = vtmp_p.tile([128, IB], f32)
                        nc.vector.tensor_copy(vts[:], pslice)
                        for jb in range(IB // 128):
                            pst = psq.tile([128, IB], f32, tag="q")
                            nc.tensor.transpose(
                                pst[:, 0:128],
                                vts[:, jb * 128:(jb + 1) * 128],
                                IDN[:],
                            )
                            nc.vector.tensor_copy(
                                VN[:, b * 16 + ib * 4 + jb, :], pst[:, 0:128]
                            )

                for c in range(5):
                    ems.append(lambda c=c: chunk(c))
                return ems

            pairs = [(b, ib) for b in range(B) for ib in range(NIB)] * reps
            for em in make_qkv_emitters(*pairs[0]):
                em()
            for idx, (b, ib) in enumerate(pairs):
                    i0 = b * T + ib * IB
                    pend = (
                        make_qkv_emitters(*pairs[idx + 1])
                        if idx + 1 < len(pairs) else []
                    )
                    # ================= attention for this i-block =============
                    o_ps = [None, None]
                    z_ps = [None, None]
                    rb = [None, None]
                    njt = 4 * ib + 4   # causal: j-tiles 0 .. 4*ib+3
                    stride = max(1, (2 * njt) // (len(pend) + 1))
                    g = 0
                    for p in range(2):          # score 1 / score 2
                        qc, kc = p, 2 + p       # chunk ids of qT and kT
                        z_ps[p] = psa.tile([128, IB], f32, tag="acc",
                                           name=f"z_ps{p}")
                        o_ps[p] = psa.tile([128, IB], f32, tag="acc",
                                           name=f"o_ps{p}")
                        for jt in range(njt):
                            r = jt - 4 * ib     # >=0 on the block-diagonal band
                            lo = 128 * r if r > 0 else 0
                            w = IB - lo
                            jq = b * T + jt * 128
                            st = psw.tile([128, 1024], f32, tag="w")
                            stv = st[:].rearrange("q (h w) -> q h w", h=2)
                            for h in range(2):
                                nc.tensor.matmul(
                                    st[:, h * IB + lo:(h + 1) * IB],
                                    QKS[h * 64:(h + 1) * 64, kc, jq:jq + 128],
                                    QKS[h * 64:(h + 1) * 64, qc, i0 + lo:i0 + IB],
                                    start=True,
                                    stop=True,
                                    tile_position=(h * 64, 0),
                                )
                            if r >= 0:
                                # additive triangular mask on the diagonal block
                                nc.vector.tensor_tensor(
                                    stv[:, :, lo:lo + 128],
                                    stv[:, :, lo:lo + 128],
                                    TRI[:, None, :].to_broadcast((128, 2, 128)),
                                    ADD,
                                )
                            pp = pp_p.tile([128, 1024], bf16, tag="pp")
                            ppv = pp[:].rearrange("q (h w) -> q h w", h=2)
                            nc.scalar.activation(
                                ppv[:, :, lo:IB], stv[:, :, lo:IB], EXP, scale=0.125
                            )
                            first = jt == 0
                            last = jt == njt - 1
                            for h in range(2):
                                nc.tensor.matmul(
                                    o_ps[p][h * 64:(h + 1) * 64, lo:IB],
                                    VN[:, b * 16 + jt, h * 64:(h + 1) * 64],
                                    pp[:, h * IB + lo:(h + 1) * IB],
                                    start=first,
                                    stop=last,
                                    tile_position=(0, h * 64),
                                    skip_group_check=True,
                                )
                            for h in range(2):
                                nc.tensor.matmul(
                                    z_ps[p][h * 64:(h + 1) * 64, lo:IB],
                                    ONE[:],
                                    pp[:, h * IB + lo:(h + 1) * IB],
                                    start=first,
                                    stop=last,
                                    tile_position=(0, h * 64),
                                    skip_group_check=True,
                                )
                            g += 1
                            if pend and g % stride == 0:
                                pend.pop(0)()
                        # normalizer for this score: 1/Z broadcast across the
                        # 64 replicated psum rows, straight off the Z bank
                        rb[p] = rb_p.tile([128, IB], f32, tag="rb",
                                          name=f"rb{p}")
                        nc.vector.reciprocal(rb[p][:], z_ps[p][:])
                        if p == 1:
                            nc.vector.tensor_scalar(
                                rb[p][:], rb[p][:], LAMC[:, 0:1], None, MUL
                            )
                    while pend:
                        pend.pop(0)()

                    # ================= combine ================================
                    t1 = tmp_p.tile([128, IB], f32, tag="t1")
                    t2 = tmp_p.tile([128, IB], f32, tag="t2")
                    nc.vector.tensor_tensor(t1[:], o_ps[0][:], rb[0][:], MUL)
                    nc.vector.tensor_tensor(t2[:], o_ps[1][:], rb[1][:], MUL)
                    OF = of_p.tile([128, IB], f32r)
                    nc.vector.tensor_tensor(OF[:], t1[:], t2[:], SUB)

                    # ================= out-projection (partial) ===============
                    for ic in range(IB // 128):
                        ysb = ys_p.tile([128, 1024], f32)
                        for nt in range(DM // 512):
                            pso = psa.tile([128, IB], f32, tag="acc",
                                           name=f"pso{nt}")
                            nc.tensor.matmul(
                                pso[:],
                                OF[:, ic * 128:(ic + 1) * 128],
                                WO[:, nt * 512:(nt + 1) * 512],
                                start=True,
                                stop=True,
                            )
                            nc.vector.tensor_copy(
                                ysb[:, nt * 512:(nt + 1) * 512], pso[:]
                            )
                        nc.sync.dma_start(
                            y[i0 + ic * 128:i0 + (ic + 1) * 128, :], ysb[:]
                        )

    nc.compile()
    _cached[key] = nc
    return nc


def _lam128(lam, core):
    hA, hB = 2 * core, 2 * core + 1
    return np.repeat(np.array([lam[hA], lam[hB]], dtype=np.float32), 64)[:, None]


def make_in_maps(x, mask, W_qkv, W_out, lam):
    x = np.asarray(x, dtype=np.float32)
    mask = np.asarray(mask, dtype=np.float32)
    W_qkv = np.asarray(W_qkv, dtype=np.float32)
    W_out = np.asarray(W_out, dtype=np.float32)
    lam = np.asarray(lam, dtype=np.float32)

    xt = np.ascontiguousarray(x.reshape(NI, DM).T)            # [1024, 4096]
    tri = np.ascontiguousarray(mask[0, 0, :128, :128].T)      # [128, 128] maskT
    Wr = W_qkv.reshape(DM, H, 5, D)
    Wo_r = W_out.reshape(H, D, DM)
    in_maps = []
    for c in range(NCORES):
        hA, hB = 2 * c, 2 * c + 1
        wq_loc = np.ascontiguousarray(
            np.concatenate(
                [Wr[:, [hA, hB], t, :].reshape(DM, 2 * D) for t in range(5)],
                axis=1,
            )
        )                                                      # [1024, 640]
        wo_loc = np.ascontiguousarray(Wo_r[[hA, hB]].reshape(2 * D, DM))  # [128,1024]
        in_maps.append(
            {
                "xt": xt,
                "wq": wq_loc,
                "wo": wo_loc,
                "tri": tri,
                "lam128": _lam128(lam, c),
            }
        )
    return in_maps


def kernel(x, mask, W_qkv, W_out, lam):
    _, _, _, run_bass_kernel_spmd, _ = _modules()
    nc = build_nc()
    in_maps = make_in_maps(x, mask, W_qkv, W_out, lam)
    res = run_bass_kernel_spmd(nc, in_maps, core_ids=list(range(NCORES)))
    parts = [res.results[c]["y"] for c in range(NCORES)]
    y = parts[0].astype(np.float64)
    for p in parts[1:]:
        y = y + p
    return y.astype(np.float32).reshape(B, T, DM)

